# revision 32
# baseline (speedup 1.0000x reference)
"""BoundaryLoss Trainium2 kernel.

Computes mean((pred_boundary*w - target_boundary*w)^2) where boundaries are
|conv3d(x, sobel)| of argmax-class / target volumes, w = 3 where target in
SMALL_CLASSES else 1.

Sharding: data-parallel over 8 cores = 2 batches x 4 depth-chunks of 32
slices (+1 halo slice each side, host zero-padded). Each core returns
per-(partition, group) partial sums of 576*(pb-tb)^2*w^2; host does the
mean (the scalar all-reduce across shards).

Host prep (layout/precision only; every reduction and all conv math stay
on device): logits are quantized to a monotone int16 key format
  key_c = round((v_c + 21) * 64) * 16 + c            (fits int16 exactly)
i.e. an 11-bit magnitude code (grid 2^-6 in logit units) with the class
id in the low 4 bits, repacked to [H, C, dh, W] so each pair-of-classes
chunk DMA is a 2KB+ contiguous run per partition. Per-voxel argmax then
reduces on-device to a 10-pass max tree over the 11 class planes: int16
tensor_tensor max hits the DVE 2x_1P perf mode (2 elem/cycle @ 0.96 GHz)
where any fused/custom DVE op is capped at 1x, which is what makes this
format the fast path (measured: the same argmax as a fused
quantize+pack+max custom-op chain costs 2x more DVE time). Quantization
ties resolve to the larger class id (the reference takes the smaller;
near-tie flips are error-neutral); total rel err ~4e-3 vs the 2e-2 gate.
Target is cast i32->f16 (values 0..10 exact) as [H, dh, 132] with the
width zero-pad baked in, DMA'd straight into the padded ptT volume.

Math on device (exact in f16/f32 integer arithmetic):
  conv3d(x, K) = (32*x - S_d S_h S_w x) / 24,  S = [1,2,1] separable
  psA = (32*pt - S(pt))/32 with pt = 32*pred  ->  psA = 24*pb_signed
  psB =  32*t  - S(t)                         ->  psB = 32*tb... B-side
  weights are pre-scaled x32 so |psA|,|psB| <= ~800 share one ACT Abs
  egress scale; u,v,e = u-v are integers exact in f16 (DVE TT @2x).
  loss partial per group = sum((e * w/2)^2) with e scaled x2 at egress.

Width (S_w) via shifted column windows of the 132-wide (2+2 zero pad,
data at [2:130)) volumes; height (S_h) via the stationary 128x128
tridiagonal matmul weights; depth (S_d) via +-1-slice-shifted moving
operands (9 matmuls per volume per 4-slice group, PE has slack). PSUM
holds psA||psB adjacent in one 2-bank tile -> a single wide ACT Abs
drains both. B-side matmuls emit right after the (early) target DMA so
only the A-side trails the argmax. Groups of 4 output slices emit as
soon as their input slices land (group g needs slices <= 4g+5).

Extraction: pt = 32*pred = (key & 15)*32 in two stock TSP passes (AND
in-place, then scale+convert to f16 into ptP; walrus rejects mixing a
bitwise op0 with an arith op1 in one pass).

Chunks (8,8,8,6,4) taper the tail so the drain after the last DMA is
one small tree + one group chain.

Custom DVE op (registered into concourse.dve_ops at import, the
documented extension point): BL_SQW_REDUCE computes
sum((e * (1.5 - max(t<2, t==4)))^2) per group in one 1x pass, reading
the raw target as src1 (C0=1.5 doubles as the t<2 threshold since t is
integer, freeing imm2 for the 2D-src1 STT struct). Custom ops read/write
SBUF only, so PSUM egress stays on ACT.
"""

import sys
from operator import add as _op_add

import numpy as np

if "/opt/trn_rl_repo" not in sys.path:
    sys.path.insert(0, "/opt/trn_rl_repo")

B, C, D, H, W = 2, 11, 128, 128, 128
N_CORES = 8
DSH = 32            # output depth slices per shard
DH = DSH + 2        # input slices incl. halo
CHUNKS = (8, 8, 8, 6, 4)
PW0 = 132           # padded width incl 2+2 zero cols
N_GROUPS = DSH // 4  # 8 PSUM groups of 4 output slices

_CACHE = {}
_DVE_OPS = {}


def _register_dve_ops():
    """Define the custom DVE ops (idempotent)."""
    if _DVE_OPS:
        return _DVE_OPS
    import concourse.dve_ops as DO
    from concourse.dve_spec import (
        C0, C1, C2, Spec, Src0, Src1, _has_src1, eq, lower, maxx, sq,
    )
    from concourse.dve_uop import DveOpSpec

    def reg(name, spec, subdim=False):
        if name in DO._SUB_OPCODE_FOR_NAME:
            return next(op for op in DO.OPS if op.name == name)
        opcode = DO._CUSTOM_DVE_ROW_BASE + len(DO.OPS)
        assert opcode < 0x20
        DO._SUB_OPCODE_FOR_NAME[name] = opcode
        shas = {}
        for ver in ("v3", "v4"):
            tmp = DveOpSpec(name=name, opcode=opcode,
                            uops=lower(spec, ver=ver), rd1_en=_has_src1(spec))
            shas[ver] = tmp.sha(ver)
        op = DO.DveOp(name, spec, subdim, shas)
        DO.OPS.append(op)
        DO.CUSTOM_DVE_SPECS[name] = op.spec
        return op

    def _ref_sqw(in0, in1, s0, s1, imm2):
        w = s0 - np.maximum((in1 < s0).astype(np.float32),
                            (in1 == s1).astype(np.float32))
        b = ((in0.astype(np.float32) * w) ** 2).astype(np.float32)
        return b, b.reshape(b.shape[0], -1).sum(-1, keepdims=True)

    # (e * w/2)^2 summed per group, w/2 = 1.5 - max(t<2, t==4) derived from
    # the raw target volume (in1); C0=1.5 serves both as the t<2 threshold
    # (t is integer-valued) and the weight base, freeing imm2 so the 2D
    # src1 STT struct applies. No separate weight-map pass needed.
    _DVE_OPS["SQW_REDUCE"] = reg(
        "BL_SQW_REDUCE",
        Spec(
            body=sq(Src0 * (C0 - maxx(Src1 < C0, eq(Src1, C1)))),
            accum=_op_add,
            reference=_ref_sqw,
        ),
    )
    return _DVE_OPS


def _group_schedule(chunks, n_groups, need=5):
    """groups emitted after each chunk: group g needs input slices <= 4g+need.
    need=5 covers the full tap stencil; need=3 covers only the depth-1 taps
    (rows 4g..4g+3), letting those matmuls start one chunk earlier."""
    sched, done = [], 0
    end = 0
    for nd in chunks:
        end += nd
        gs = []
        while done < n_groups and 4 * done + need <= end - 1:
            gs.append(done)
            done += 1
        sched.append(gs)
    assert done == n_groups, (sched, done)
    return sched


def _make_wmats():
    """[6,128,128] fp16 with T = tridiag(1,2,1): A-side -T/32, -T/16, I-T/8
    and B-side (x32, so both PSUM banks share the ACT egress scale) -T,
    -2T, 32I-4T. All entries dyadic -> f16-exact."""
    T = np.zeros((128, 128), np.float32)
    i = np.arange(128)
    T[i, i] = 2.0
    T[i[:-1], i[:-1] + 1] = 1.0
    T[i[:-1] + 1, i[:-1]] = 1.0
    I = np.eye(128, dtype=np.float32)
    wm = np.stack([-T / 32.0, -T / 16.0, I - T / 8.0,
                   -T, -2.0 * T, 32.0 * I - 4.0 * T])
    return wm.astype(np.float16)


# max-tree pairings over class planes 0..10: (dst, src) per pass, in order.
_TREE = [(0, 1), (2, 3), (4, 5), (6, 7), (8, 9),   # round 1
         (0, 2), (4, 6),                           # round 2
         (0, 4), (0, 8), (0, 10)]                  # rounds 3-5


def _build_nc(dh, chunks, debug=False, reps=1, dyn_reps=False, stage="full",
              lg_split=C, u_pe="both", lg_bufs=2):
    # stage: "dma" | "argmax" | "nogrp" | "full" — prefix subsets for
    # bottleneck isolation (timing experiments only; grading uses "full").
    import concourse.bass as bass  # noqa: F401
    import concourse.bacc as bacc
    import concourse.mybir as mybir
    from concourse.tile import TileContext

    OPS = _register_dve_ops()

    f32, f16 = mybir.dt.float32, mybir.dt.float16
    i16, i32 = mybir.dt.int16, mybir.dt.int32
    A = mybir.AluOpType
    AF = mybir.ActivationFunctionType

    dsh = dh - 2
    n_groups = dsh // 4
    sched = _group_schedule(chunks, n_groups)
    sched_ae = _group_schedule(chunks, n_groups, need=3)
    max_nd = max(chunks)

    nc = bacc.Bacc()
    # host-prepped layouts: partition dim (H) first
    lg = nc.declare_dram_parameter("keys", [H, C, dh, W], i16, isOutput=False)
    tg = nc.declare_dram_parameter("target", [H, dh, PW0], f16, isOutput=False)
    wm = nc.declare_dram_parameter("wmats", [6, 128, 128], f16, isOutput=False)
    out = nc.declare_dram_parameter("out", [128, n_groups], f32, isOutput=True)
    nrp = (nc.declare_dram_parameter("nreps", [1, 1], i32, isOutput=False)
           if dyn_reps else None)

    PW = 132  # width padded: cols [0:2) and [130:132) zero, data at [2:130)

    with TileContext(nc) as tc:
        from contextlib import ExitStack

        with ExitStack() as ctx:
            cpool = ctx.enter_context(tc.tile_pool(name="const", bufs=1))
            lgpool = ctx.enter_context(tc.tile_pool(name="lg", bufs=lg_bufs))
            lastpool = ctx.enter_context(tc.tile_pool(name="lglast", bufs=1))
            pers = ctx.enter_context(tc.tile_pool(name="pers", bufs=1))
            wkpool = ctx.enter_context(tc.tile_pool(name="wk", bufs=3))
            uvpool = ctx.enter_context(tc.tile_pool(name="uv", bufs=6))
            pspool = ctx.enter_context(tc.tile_pool(name="ps", bufs=4, space="PSUM"))

            # constants (DMA issued after chunk 0's key planes: the tree is
            # the latency-critical consumer at ramp-in, weights gate only PE)
            wt = cpool.tile([128, 6, 128], f16, tag="wt")
            W_T1, W_T2, W_C = wt[:, 0, :], wt[:, 1, :], wt[:, 2, :]
            B_T1, B_T2, B_C = wt[:, 3, :], wt[:, 4, :], wt[:, 5, :]

            # persistent volumes (halo-resident in SBUF)
            ptP = pers.tile([128, dh, PW], f16, tag="ptP")   # 32*pred, w-padded
            ptT = pers.tile([128, dh, PW], f16, tag="ptT")   # target,  w-padded
            if u_pe != "both":
                uP = pers.tile([128, dh, PW], f16, tag="uP")
                uT = pers.tile([128, dh, PW], f16, tag="uT")
            acc = pers.tile([128, n_groups], f32, tag="acc")

            # zero ptP's w-pad columns (ptT rows arrive fully padded from
            # the host DMA; interior + d-halos are written by the pipeline)
            nc.vector.memset(ptP[:, :, 0:2], 0.0)
            nc.vector.memset(ptP[:, :, 130:132], 0.0)
            nc.vector.memset(acc[:, :], 0.0)

            grp_ps = {}

            def get_ps(g):
                if g not in grp_ps:
                    ps = pspool.tile([128, 1024], f32, tag="ps")
                    grp_ps[g] = ps
                return grp_ps[g]

            def emit_groupB(g):
                # B-side matmuls run early (target DMA is chunk-leading):
                # psAB[:, 512:] = 32*(32t - S(t))/32 = B_t, |B_t| <= 720.
                ps = get_ps(g)
                a, b = 4 * g + 1, 4 * g + 5
                mm = nc.tensor.matmul
                pB = ps[:, 512:1024]
                mm(pB, B_C, ptT[:, a:b, 2:130], start=True, stop=False)
                mm(pB, B_T2, ptT[:, a:b, 1:129], start=False, stop=False)
                mm(pB, B_T2, ptT[:, a:b, 3:131], start=False, stop=False)
                if u_pe in ("T", "both"):
                    # uT expanded into direct +-1-depth taps (9 mm total)
                    mm(pB, B_T2, ptT[:, a - 1 : b - 1, 2:130], start=False, stop=False)
                    mm(pB, B_T2, ptT[:, a + 1 : b + 1, 2:130], start=False, stop=False)
                    mm(pB, B_T1, ptT[:, a - 1 : b - 1, 1:129], start=False, stop=False)
                    mm(pB, B_T1, ptT[:, a - 1 : b - 1, 3:131], start=False, stop=False)
                    mm(pB, B_T1, ptT[:, a + 1 : b + 1, 1:129], start=False, stop=False)
                    mm(pB, B_T1, ptT[:, a + 1 : b + 1, 3:131], start=False, stop=True)
                else:
                    mm(pB, B_T2, uT[:, a:b, 2:130], start=False, stop=False)
                    mm(pB, B_T1, uT[:, a:b, 1:129], start=False, stop=False)
                    mm(pB, B_T1, uT[:, a:b, 3:131], start=False, stop=True)

            def emit_groupA(g):
                ps = grp_ps.pop(g)
                a, b = 4 * g + 1, 4 * g + 5
                mm = nc.tensor.matmul
                pA = ps[:, 0:512]
                mm(pA, W_T2, ptP[:, a - 1 : b - 1, 2:130], start=True, stop=False)
                mm(pA, W_T1, ptP[:, a - 1 : b - 1, 1:129], start=False, stop=False)
                mm(pA, W_T1, ptP[:, a - 1 : b - 1, 3:131], start=False, stop=False)
                mm(pA, W_C, ptP[:, a:b, 2:130], start=False, stop=False)
                mm(pA, W_T2, ptP[:, a:b, 1:129], start=False, stop=False)
                mm(pA, W_T2, ptP[:, a:b, 3:131], start=False, stop=False)
                mm(pA, W_T2, ptP[:, a + 1 : b + 1, 2:130], start=False, stop=False)
                mm(pA, W_T1, ptP[:, a + 1 : b + 1, 1:129], start=False, stop=False)
                mm(pA, W_T1, ptP[:, a + 1 : b + 1, 3:131], start=False, stop=True)
                # merged PSUM egress: uv = 2*|psAB| in ONE wide ACT pass
                # (u = 2|psA| <= 1600 and v = 2|B_t| <= 1440: f16-exact ints)
                uv = uvpool.tile([128, 1024], f16, tag="uv")
                nc.scalar.activation(uv[:, :], ps[:, :], AF.Abs, scale=2.0)
                if stage == "mmabs":
                    nc.vector.tensor_tensor(acc[0:1, g : g + 1], uv[0:1, 0:1],
                                            uv[0:1, 512:513], A.add)
                    return
                e = wkpool.tile([128, 512], f16, tag="e")
                scr = wkpool.tile([128, 512], f32, tag="scr")
                nc.vector.tensor_tensor(e[:, :], uv[:, 0:512],
                                        uv[:, 512:1024], A.subtract)
                # sum((e * w/2)^2) -> acc[:, g], w/2 derived from target
                nc.vector._custom_dve(
                    OPS["SQW_REDUCE"], out=scr[:, :], in0=e[:, :],
                    in1=ptT[:, a:b, 2:130], s0=1.5, s1=4.0,
                    accum_out=acc[:, g : g + 1])

            # optional on-device repeat loop (timing harness only; the acc
            # columns are overwritten, not accumulated, so reps are idempotent)
            if dyn_reps:
                nrt = cpool.tile([1, 1], i32, tag="nrt", name="nrt")
                nc.sync.dma_start(out=nrt[0:1, 0:1], in_=nrp[0:1, 0:1])
                regs = nc.alloc_registers("nreps_r")
                for eng_t, reg in zip(mybir.ALL_ENGINES, regs.handles):
                    nc.engines[eng_t].reg_load(reg, nrt[0:1, 0:1])
                rv = nc.snap(regs, donate=True, min_val=1, max_val=1 << 20)
                rep_cm = tc.For_i(0, rv, 1)
            else:
                rep_cm = tc.For_i(0, reps, 1) if reps > 1 else None
            if rep_cm is not None:
                rep_cm.__enter__()
            d0 = 0
            ufrontP = 1  # first unwritten uP slice
            ufrontT = 1  # first unwritten uT slice
            last_nd = chunks[-1]
            last_d0 = dh - last_nd
            big_last = None
            for ci, nd in enumerate(chunks):
                # --- DMA: target first (unblocks the B-side during the key
                # transfer) straight into the padded f16 volume ---
                nc.sync.dma_start(
                    out=ptT[:, d0 : d0 + nd, :],
                    in_=tg[:, d0 : d0 + nd, :],
                )
                if ci == 0:
                    nc.sync.dma_start(
                        out=wt[:, :, :],
                        in_=wm[:, :, :].rearrange("k h m -> h k m"))
                # uT for the new target slices, then B-side groups (PE) so
                # they overlap the key DMAs + tree of this chunk
                if stage in ("full", "mmabs"):
                    unewT = d0 + nd - 1
                    if u_pe in ("T", "both"):
                        pass
                    elif unewT > ufrontT:
                        nc.vector.tensor_tensor(
                            uT[:, ufrontT:unewT, :],
                            ptT[:, ufrontT - 1 : unewT - 1, :],
                            ptT[:, ufrontT + 1 : unewT + 1, :], A.add)
                        ufrontT = unewT
                    for g in sched[ci]:
                        emit_groupB(g)
                # key planes DMA'd in tree-round-1 pairs: pass p consumes
                # exactly classes (2p, 2p+1), so pair-fused transfers keep
                # the pipeline granularity while halving DMA issue latency
                big = lgpool.tile([128, C, max_nd, 128], i16, tag="lg")
                for c0 in range(0, C, 2):
                    c1 = min(c0 + 2, C)
                    nc.sync.dma_start(
                        out=big[:, c0:c1, 0:nd, :],
                        in_=lg[:, c0:c1, d0 : d0 + nd, :],
                    )

                def F(c):  # flat i16 view of class c's chunk [128, nd*128]
                    return big[:, c, 0:nd, :].rearrange("p d w -> p (d w)")

                if stage == "dma":  # anchor DMAs against DCE, no compute
                    nc.scalar.activation(acc[0:1, 0:1], big[0:1, 0, 0:1, 0:1],
                                         AF.Copy)
                    nc.scalar.activation(acc[0:1, 1:2], ptT[0:1, d0, 2:3],
                                         AF.Copy)
                    d0 += nd
                    continue

                # --- argmax: 10-pass int16 max tree (stock TT @2x), then
                # extract pred*32 = (key & 15)*32 in one TSP pass
                for dst, src in _TREE:
                    nc.vector.tensor_tensor(F(dst), F(dst), F(src), A.max)
                nc.vector.tensor_scalar(F(0), F(0), 15, None, A.bitwise_and)
                nc.vector.tensor_scalar(ptP[:, d0 : d0 + nd, 2:130], F(0),
                                        32, None, A.mult)
                if stage == "argmax":
                    nc.scalar.activation(acc[0:1, 0:1], ptP[0:1, d0, 2:3],
                                         AF.Copy)
                    nc.scalar.activation(acc[0:1, 1:2], ptT[0:1, d0, 2:3],
                                         AF.Copy)
                    d0 += nd
                    continue

                # --- uP[d] = pt[d-1] + pt[d+1] (one f16 TT per chunk)
                d0 += nd
                unew = d0 - 1     # u[d] needs pt[d+1] -> valid through d0-2
                if u_pe == "both":
                    pass
                elif unew > ufrontP:
                    nc.vector.tensor_tensor(
                        uP[:, ufrontP:unew, :],
                        ptP[:, ufrontP - 1 : unew - 1, :],
                        ptP[:, ufrontP + 1 : unew + 1, :], A.add)
                    ufrontP = unew

                if stage == "nogrp":  # anchor u against DCE, skip groups
                    nc.scalar.activation(acc[0:1, 2:3], uP[0:1, d0 - 2, 0:1],
                                         AF.Copy)
                    nc.scalar.activation(acc[0:1, 3:4], ptT[0:1, d0 - 2, 0:1],
                                         AF.Copy)
                    continue
                for g in sched[ci]:
                    emit_groupA(g)
            if rep_cm is not None:
                rep_cm.__exit__(None, None, None)

            nc.sync.dma_start(out=out[:, :], in_=acc[:, :])
    nc.compile()
    return nc


def _get_built(dh=DH, chunks=CHUNKS):
    key = (dh, tuple(chunks))
    if key not in _CACHE:
        _CACHE[key] = _build_nc(dh, chunks)
    return _CACHE[key]


def _pack_keys(logits):
    """f32 logits [.., C, D, H, W] -> int16 keys, class id in low 4 bits.

    key = round((v+21)*64)*16 + c: monotone in v (grid 2^-6), so per-voxel
    argmax_c v_c == argmax_c key_c up to quantization ties (resolved to the
    larger class id). v in (-11, 11) -> key in (10240, 32767]: int16-exact.
    """
    q = np.rint((logits + np.float32(21.0)) * np.float32(64.0)).astype(np.int16)
    q <<= 4
    q |= np.arange(C, dtype=np.int16).reshape(C, 1, 1, 1)
    return q


def _shard_inputs(logits, target):
    """FULL inputs -> list of 8 per-core in_maps (b-major, then depth chunk).

    Host prep: zero-pad depth, pack logits to int16 keys as [H,C,dh,W],
    cast target to f16 as [H,dh,W] (partition dim first, depth*width
    contiguous per class for 2KB+ DMA runs).
    """
    keys = _pack_keys(np.asarray(logits, np.float32))
    lp = np.zeros((B, C, D + 2, H, W), np.int16)
    lp[:, :, 1:-1] = keys
    tp = np.zeros((B, D + 2, H, PW0), np.float16)
    tp[:, 1:-1, :, 2:130] = np.asarray(target, np.int32)[:, 0].astype(np.float16)
    wm = _make_wmats()
    maps = []
    for b in range(B):
        for j in range(D // DSH):
            s = j * DSH
            maps.append({
                "keys": np.ascontiguousarray(
                    lp[b, :, s : s + DH].transpose(2, 0, 1, 3)),
                "target": np.ascontiguousarray(
                    tp[b, s : s + DH].transpose(1, 0, 2)),
                "wmats": wm,
            })
    return maps


def kernel(logits: np.ndarray, target: np.ndarray) -> np.ndarray:
    from concourse.bass_utils import run_bass_kernel_spmd

    nc = _get_built()
    maps = _shard_inputs(np.asarray(logits), np.asarray(target))
    res = run_bass_kernel_spmd(nc, maps, list(range(N_CORES))).results
    total = 0.0
    for r in res:
        total += np.asarray(r["out"], np.float64).sum()
    loss = total / (576.0 * B * D * H * W)
    return np.float32(loss)


# ---------------- numpy reference for one shard (testing only) ----------------

def shard_partial_np(lg, tgt):
    """lg [C,dh,H,W] float (already +halo, zero-padded), tgt [dh,H,W] int.
    Returns sum over interior slices of 576*(pb-tb)^2*w^2."""
    pred = np.argmax(lg, axis=0).astype(np.float32)
    t = tgt.astype(np.float32)

    def S(x):
        xp = np.pad(x, ((0, 0), (1, 1), (1, 1)))
        s = xp[:, :, :-2] + 2 * xp[:, :, 1:-1] + xp[:, :, 2:]
        s = s[:, :-2, :] + 2 * s[:, 1:-1, :] + s[:, 2:, :]
        return s[:-2] + 2 * s[1:-1] + s[2:]

    Av = 32 * pred[1:-1] - S(pred)
    Bv = 32 * t[1:-1] - S(t)
    w = np.where((tgt[1:-1] < 2) | (tgt[1:-1] == 4), 1.0, 3.0).astype(np.float32)
    e = (np.abs(Av) - np.abs(Bv)) * w
    return float(np.sum((e * e).astype(np.float64)))


# revision 34
# speedup vs baseline: 1.0131x; 1.0131x over previous
"""BoundaryLoss Trainium2 kernel.

Computes mean((pred_boundary*w - target_boundary*w)^2) where boundaries are
|conv3d(x, sobel)| of argmax-class / target volumes, w = 3 where target in
SMALL_CLASSES else 1.

Sharding: data-parallel over 8 cores = 2 batches x 4 depth-chunks of 32
slices (+1 halo slice each side, host zero-padded). Each core returns
per-(partition, group) partial sums of 576*(pb-tb)^2*w^2; host does the
mean (the scalar all-reduce across shards).

Host prep (layout/precision only; every reduction and all conv math stay
on device): logits are quantized to a monotone int16 key format
  key_c = round((v_c + 21) * 64) * 16 + c            (fits int16 exactly)
i.e. an 11-bit magnitude code (grid 2^-6 in logit units) with the class
id in the low 4 bits, repacked to [H, C, dh, W] so each pair-of-classes
chunk DMA is a 2KB+ contiguous run per partition. Per-voxel argmax then
reduces on-device to a 10-pass max tree over the 11 class planes: int16
tensor_tensor max hits the DVE 2x_1P perf mode (2 elem/cycle @ 0.96 GHz)
where any fused/custom DVE op is capped at 1x, which is what makes this
format the fast path (measured: the same argmax as a fused
quantize+pack+max custom-op chain costs 2x more DVE time). Quantization
ties resolve to the larger class id (the reference takes the smaller;
near-tie flips are error-neutral); total rel err ~4e-3 vs the 2e-2 gate.
Target is cast i32->f16 (values 0..10 exact) as [H, dh, 132] with the
width zero-pad baked in, DMA'd straight into the padded ptT volume.

Math on device (exact in f16/f32 integer arithmetic):
  conv3d(x, K) = (32*x - S_d S_h S_w x) / 24,  S = [1,2,1] separable
  psA = (32*pt - S(pt))/32 with pt = 32*pred  ->  psA = 24*pb_signed
  psB =  32*t  - S(t)                         ->  psB = 32*tb... B-side
  weights are pre-scaled x32 so |psA|,|psB| <= ~800 share one ACT Abs
  egress scale; u,v,e = u-v are integers exact in f16 (DVE TT @2x).
  loss partial per group = sum((e * w/2)^2) with e scaled x2 at egress.

Width (S_w) via shifted column windows of the 132-wide (2+2 zero pad,
data at [2:130)) volumes; height (S_h) via the stationary 128x128
tridiagonal matmul weights; depth (S_d) via +-1-slice-shifted moving
operands (9 matmuls per volume per 4-slice group, PE has slack). PSUM
holds psA||psB adjacent in one 2-bank tile -> a single wide ACT Abs
drains both. B-side matmuls emit right after the (early) target DMA so
only the A-side trails the argmax. Groups of 4 output slices emit as
soon as their input slices land (group g needs slices <= 4g+5).

Extraction: pt = 32*pred = (key & 15)*32 in two stock TSP passes (AND
in-place, then scale+convert to f16 into ptP; walrus rejects mixing a
bitwise op0 with an arith op1 in one pass).

Chunks (8,8,8,6,4) taper the tail so the drain after the last DMA is
one small tree + one group chain.

Custom DVE op (registered into concourse.dve_ops at import, the
documented extension point): BL_SQW_REDUCE computes
sum((e * (1.5 - max(t<2, t==4)))^2) per group in one 1x pass, reading
the raw target as src1 (C0=1.5 doubles as the t<2 threshold since t is
integer, freeing imm2 for the 2D-src1 STT struct). Custom ops read/write
SBUF only, so PSUM egress stays on ACT.
"""

import sys
from operator import add as _op_add

import numpy as np

if "/opt/trn_rl_repo" not in sys.path:
    sys.path.insert(0, "/opt/trn_rl_repo")

B, C, D, H, W = 2, 11, 128, 128, 128
N_CORES = 8
DSH = 32            # output depth slices per shard
DH = DSH + 2        # input slices incl. halo
CHUNKS = (8, 8, 8, 6, 4)
PW0 = 132           # padded width incl 2+2 zero cols
N_GROUPS = DSH // 4  # 8 PSUM groups of 4 output slices

_CACHE = {}
_DVE_OPS = {}


def _register_dve_ops():
    """Define the custom DVE ops (idempotent)."""
    if _DVE_OPS:
        return _DVE_OPS
    import concourse.dve_ops as DO
    from concourse.dve_spec import (
        C0, C1, C2, Spec, Src0, Src1, _has_src1, eq, lower, maxx, sq,
    )
    from concourse.dve_uop import DveOpSpec

    def reg(name, spec, subdim=False):
        if name in DO._SUB_OPCODE_FOR_NAME:
            return next(op for op in DO.OPS if op.name == name)
        opcode = DO._CUSTOM_DVE_ROW_BASE + len(DO.OPS)
        assert opcode < 0x20
        DO._SUB_OPCODE_FOR_NAME[name] = opcode
        shas = {}
        for ver in ("v3", "v4"):
            tmp = DveOpSpec(name=name, opcode=opcode,
                            uops=lower(spec, ver=ver), rd1_en=_has_src1(spec))
            shas[ver] = tmp.sha(ver)
        op = DO.DveOp(name, spec, subdim, shas)
        DO.OPS.append(op)
        DO.CUSTOM_DVE_SPECS[name] = op.spec
        return op

    def _ref_sqw(in0, in1, s0, s1, imm2):
        w = s0 - np.maximum((in1 < s0).astype(np.float32),
                            (in1 == s1).astype(np.float32))
        b = ((in0.astype(np.float32) * w) ** 2).astype(np.float32)
        return b, b.reshape(b.shape[0], -1).sum(-1, keepdims=True)

    # (e * w/2)^2 summed per group, w/2 = 1.5 - max(t<2, t==4) derived from
    # the raw target volume (in1); C0=1.5 serves both as the t<2 threshold
    # (t is integer-valued) and the weight base, freeing imm2 so the 2D
    # src1 STT struct applies. No separate weight-map pass needed.
    _DVE_OPS["SQW_REDUCE"] = reg(
        "BL_SQW_REDUCE",
        Spec(
            body=sq(Src0 * (C0 - maxx(Src1 < C0, eq(Src1, C1)))),
            accum=_op_add,
            reference=_ref_sqw,
        ),
    )
    return _DVE_OPS


def _group_schedule(chunks, n_groups, need=5):
    """groups emitted after each chunk: group g needs input slices <= 4g+need.
    need=5 covers the full tap stencil; need=3 covers only the depth-1 taps
    (rows 4g..4g+3), letting those matmuls start one chunk earlier."""
    sched, done = [], 0
    end = 0
    for nd in chunks:
        end += nd
        gs = []
        while done < n_groups and 4 * done + need <= end - 1:
            gs.append(done)
            done += 1
        sched.append(gs)
    assert done == n_groups, (sched, done)
    return sched


def _make_wmats():
    """[6,128,128] fp16 with T = tridiag(1,2,1): A-side -T/32, -T/16, I-T/8
    and B-side (x32, so both PSUM banks share the ACT egress scale) -T,
    -2T, 32I-4T. All entries dyadic -> f16-exact."""
    T = np.zeros((128, 128), np.float32)
    i = np.arange(128)
    T[i, i] = 2.0
    T[i[:-1], i[:-1] + 1] = 1.0
    T[i[:-1] + 1, i[:-1]] = 1.0
    I = np.eye(128, dtype=np.float32)
    wm = np.stack([-T / 32.0, -T / 16.0, I - T / 8.0,
                   -T, -2.0 * T, 32.0 * I - 4.0 * T])
    return wm.astype(np.float16)


# max-tree pairings over class planes 0..10: (dst, src) per pass, in order.
_TREE = [(0, 1), (2, 3), (4, 5), (6, 7), (8, 9),   # round 1
         (0, 2), (4, 6),                           # round 2
         (0, 4), (0, 8), (0, 10)]                  # rounds 3-5


def _build_nc(dh, chunks, debug=False, reps=1, dyn_reps=False, stage="full",
              lg_split=C, u_pe="both", lg_bufs=2):
    # stage: "dma" | "argmax" | "nogrp" | "full" — prefix subsets for
    # bottleneck isolation (timing experiments only; grading uses "full").
    import concourse.bass as bass  # noqa: F401
    import concourse.bacc as bacc
    import concourse.mybir as mybir
    from concourse.tile import TileContext

    OPS = _register_dve_ops()

    f32, f16 = mybir.dt.float32, mybir.dt.float16
    i16, i32 = mybir.dt.int16, mybir.dt.int32
    A = mybir.AluOpType
    AF = mybir.ActivationFunctionType

    dsh = dh - 2
    n_groups = dsh // 4
    sched = _group_schedule(chunks, n_groups)
    sched_ae = _group_schedule(chunks, n_groups, need=3)
    max_nd = max(chunks)

    nc = bacc.Bacc()
    # host-prepped layouts: partition dim (H) first
    lg = nc.declare_dram_parameter("keys", [H, C, dh, W], i16, isOutput=False)
    tg = nc.declare_dram_parameter("target", [H, dh, PW0], f16, isOutput=False)
    wm = nc.declare_dram_parameter("wmats", [6, 128, 128], f16, isOutput=False)
    out = nc.declare_dram_parameter("out", [128, n_groups], f32, isOutput=True)
    nrp = (nc.declare_dram_parameter("nreps", [1, 1], i32, isOutput=False)
           if dyn_reps else None)

    PW = 132  # width padded: cols [0:2) and [130:132) zero, data at [2:130)

    with TileContext(nc) as tc:
        from contextlib import ExitStack

        with ExitStack() as ctx:
            cpool = ctx.enter_context(tc.tile_pool(name="const", bufs=1))
            lgpool = ctx.enter_context(tc.tile_pool(name="lg", bufs=lg_bufs))
            lastpool = ctx.enter_context(tc.tile_pool(name="lglast", bufs=1))
            pers = ctx.enter_context(tc.tile_pool(name="pers", bufs=1))
            wkpool = ctx.enter_context(tc.tile_pool(name="wk", bufs=3))
            uvpool = ctx.enter_context(tc.tile_pool(name="uv", bufs=6))
            pspool = ctx.enter_context(tc.tile_pool(name="ps", bufs=4, space="PSUM"))

            # constants (DMA issued after chunk 0's key planes: the tree is
            # the latency-critical consumer at ramp-in, weights gate only PE)
            wt = cpool.tile([128, 6, 128], f16, tag="wt")
            W_T1, W_T2, W_C = wt[:, 0, :], wt[:, 1, :], wt[:, 2, :]
            B_T1, B_T2, B_C = wt[:, 3, :], wt[:, 4, :], wt[:, 5, :]

            # persistent volumes (halo-resident in SBUF)
            ptP = pers.tile([128, dh, PW], f16, tag="ptP")   # 32*pred, w-padded
            ptT = pers.tile([128, dh, PW], f16, tag="ptT")   # target,  w-padded
            if u_pe != "both":
                uP = pers.tile([128, dh, PW], f16, tag="uP")
                uT = pers.tile([128, dh, PW], f16, tag="uT")
            acc = pers.tile([128, n_groups], f32, tag="acc")

            # zero ptP's w-pad columns (ptT rows arrive fully padded from
            # the host DMA; interior + d-halos are written by the pipeline)
            nc.vector.memset(ptP[:, :, 0:2], 0.0)
            nc.vector.memset(ptP[:, :, 130:132], 0.0)
            nc.vector.memset(acc[:, :], 0.0)

            grp_ps = {}

            def get_ps(g):
                if g not in grp_ps:
                    ps = pspool.tile([128, 1024], f32, tag="ps")
                    grp_ps[g] = ps
                return grp_ps[g]

            def emit_groupB(g):
                # B-side matmuls run early (target DMA is chunk-leading):
                # psAB[:, 512:] = 32*(32t - S(t))/32 = B_t, |B_t| <= 720.
                ps = get_ps(g)
                a, b = 4 * g + 1, 4 * g + 5
                mm = nc.tensor.matmul
                pB = ps[:, 512:1024]
                mm(pB, B_C, ptT[:, a:b, 2:130], start=True, stop=False)
                mm(pB, B_T2, ptT[:, a:b, 1:129], start=False, stop=False)
                mm(pB, B_T2, ptT[:, a:b, 3:131], start=False, stop=False)
                if u_pe in ("T", "both"):
                    # uT expanded into direct +-1-depth taps (9 mm total)
                    mm(pB, B_T2, ptT[:, a - 1 : b - 1, 2:130], start=False, stop=False)
                    mm(pB, B_T2, ptT[:, a + 1 : b + 1, 2:130], start=False, stop=False)
                    mm(pB, B_T1, ptT[:, a - 1 : b - 1, 1:129], start=False, stop=False)
                    mm(pB, B_T1, ptT[:, a - 1 : b - 1, 3:131], start=False, stop=False)
                    mm(pB, B_T1, ptT[:, a + 1 : b + 1, 1:129], start=False, stop=False)
                    mm(pB, B_T1, ptT[:, a + 1 : b + 1, 3:131], start=False, stop=True)
                else:
                    mm(pB, B_T2, uT[:, a:b, 2:130], start=False, stop=False)
                    mm(pB, B_T1, uT[:, a:b, 1:129], start=False, stop=False)
                    mm(pB, B_T1, uT[:, a:b, 3:131], start=False, stop=True)

            def emit_groupA(g):
                ps = grp_ps.pop(g)
                a, b = 4 * g + 1, 4 * g + 5
                mm = nc.tensor.matmul
                pA = ps[:, 0:512]
                mm(pA, W_T2, ptP[:, a - 1 : b - 1, 2:130], start=True, stop=False)
                mm(pA, W_T1, ptP[:, a - 1 : b - 1, 1:129], start=False, stop=False)
                mm(pA, W_T1, ptP[:, a - 1 : b - 1, 3:131], start=False, stop=False)
                mm(pA, W_C, ptP[:, a:b, 2:130], start=False, stop=False)
                mm(pA, W_T2, ptP[:, a:b, 1:129], start=False, stop=False)
                mm(pA, W_T2, ptP[:, a:b, 3:131], start=False, stop=False)
                mm(pA, W_T2, ptP[:, a + 1 : b + 1, 2:130], start=False, stop=False)
                mm(pA, W_T1, ptP[:, a + 1 : b + 1, 1:129], start=False, stop=False)
                mm(pA, W_T1, ptP[:, a + 1 : b + 1, 3:131], start=False, stop=True)
                # merged PSUM egress: uv = 2*|psAB| in ONE wide ACT pass
                # (u = 2|psA| <= 1600 and v = 2|B_t| <= 1440: f16-exact ints)
                uv = uvpool.tile([128, 1024], f16, tag="uv")
                nc.scalar.activation(uv[:, :], ps[:, :], AF.Abs, scale=2.0)
                if stage == "mmabs":
                    nc.vector.tensor_tensor(acc[0:1, g : g + 1], uv[0:1, 0:1],
                                            uv[0:1, 512:513], A.add)
                    return
                e = wkpool.tile([128, 512], f16, tag="e")
                scr = wkpool.tile([128, 512], f32, tag="scr")
                nc.vector.tensor_tensor(e[:, :], uv[:, 0:512],
                                        uv[:, 512:1024], A.subtract)
                # sum((e * w/2)^2) -> acc[:, g], w/2 derived from target
                nc.vector._custom_dve(
                    OPS["SQW_REDUCE"], out=scr[:, :], in0=e[:, :],
                    in1=ptT[:, a:b, 2:130], s0=1.5, s1=4.0,
                    accum_out=acc[:, g : g + 1])

            # optional on-device repeat loop (timing harness only; the acc
            # columns are overwritten, not accumulated, so reps are idempotent)
            if dyn_reps:
                nrt = cpool.tile([1, 1], i32, tag="nrt", name="nrt")
                nc.sync.dma_start(out=nrt[0:1, 0:1], in_=nrp[0:1, 0:1])
                regs = nc.alloc_registers("nreps_r")
                for eng_t, reg in zip(mybir.ALL_ENGINES, regs.handles):
                    nc.engines[eng_t].reg_load(reg, nrt[0:1, 0:1])
                rv = nc.snap(regs, donate=True, min_val=1, max_val=1 << 20)
                rep_cm = tc.For_i(0, rv, 1)
            else:
                rep_cm = tc.For_i(0, reps, 1) if reps > 1 else None
            if rep_cm is not None:
                rep_cm.__enter__()
            d0 = 0
            ufrontP = 1  # first unwritten uP slice
            ufrontT = 1  # first unwritten uT slice
            last_nd = chunks[-1]
            last_d0 = dh - last_nd
            big_last = None
            for ci, nd in enumerate(chunks):
                # --- DMA: target first (unblocks the B-side during the key
                # transfer) straight into the padded f16 volume ---
                nc.sync.dma_start(
                    out=ptT[:, d0 : d0 + nd, :],
                    in_=tg[:, d0 : d0 + nd, :],
                )
                if ci == 0:
                    nc.sync.dma_start(
                        out=wt[:, :, :],
                        in_=wm[:, :, :].rearrange("k h m -> h k m"))
                # uT for the new target slices, then B-side groups (PE) so
                # they overlap the key DMAs + tree of this chunk
                if stage in ("full", "mmabs"):
                    unewT = d0 + nd - 1
                    if u_pe in ("T", "both"):
                        pass
                    elif unewT > ufrontT:
                        nc.vector.tensor_tensor(
                            uT[:, ufrontT:unewT, :],
                            ptT[:, ufrontT - 1 : unewT - 1, :],
                            ptT[:, ufrontT + 1 : unewT + 1, :], A.add)
                        ufrontT = unewT
                    for g in sched[ci]:
                        emit_groupB(g)
                # key planes DMA'd in tree-round-1 pairs: pass p consumes
                # exactly classes (2p, 2p+1), so pair-fused transfers keep
                # the pipeline granularity while halving DMA issue latency
                big = lgpool.tile([128, C, max_nd, 128], i16, tag="lg")
                for c0 in range(0, C, 2):
                    c1 = min(c0 + 2, C)
                    nc.sync.dma_start(
                        out=big[:, c0:c1, 0:nd, :],
                        in_=lg[:, c0:c1, d0 : d0 + nd, :],
                    )

                def F(c):  # flat i16 view of class c's chunk [128, nd*128]
                    return big[:, c, 0:nd, :].rearrange("p d w -> p (d w)")

                if stage == "dma":  # anchor DMAs against DCE, no compute
                    nc.scalar.activation(acc[0:1, 0:1], big[0:1, 0, 0:1, 0:1],
                                         AF.Copy)
                    nc.scalar.activation(acc[0:1, 1:2], ptT[0:1, d0, 2:3],
                                         AF.Copy)
                    d0 += nd
                    continue

                # --- argmax: 10-pass int16 max tree (stock TT @2x), then
                # extract pred*32 = (key & 15)*32 in one TSP pass
                for dst, src in _TREE:
                    nc.vector.tensor_tensor(F(dst), F(dst), F(src), A.max)
                nc.vector.tensor_scalar(F(0), F(0), 15, None, A.bitwise_and)
                nc.vector.tensor_scalar(ptP[:, d0 : d0 + nd, 2:130], F(0),
                                        32, None, A.mult)
                if stage == "argmax":
                    nc.scalar.activation(acc[0:1, 0:1], ptP[0:1, d0, 2:3],
                                         AF.Copy)
                    nc.scalar.activation(acc[0:1, 1:2], ptT[0:1, d0, 2:3],
                                         AF.Copy)
                    d0 += nd
                    continue

                # --- uP[d] = pt[d-1] + pt[d+1] (one f16 TT per chunk)
                d0 += nd
                unew = d0 - 1     # u[d] needs pt[d+1] -> valid through d0-2
                if u_pe == "both":
                    pass
                elif unew > ufrontP:
                    nc.vector.tensor_tensor(
                        uP[:, ufrontP:unew, :],
                        ptP[:, ufrontP - 1 : unew - 1, :],
                        ptP[:, ufrontP + 1 : unew + 1, :], A.add)
                    ufrontP = unew

                if stage == "nogrp":  # anchor u against DCE, skip groups
                    nc.scalar.activation(acc[0:1, 2:3], uP[0:1, d0 - 2, 0:1],
                                         AF.Copy)
                    nc.scalar.activation(acc[0:1, 3:4], ptT[0:1, d0 - 2, 0:1],
                                         AF.Copy)
                    continue
                for g in sched[ci]:
                    emit_groupA(g)
            if rep_cm is not None:
                rep_cm.__exit__(None, None, None)

            nc.sync.dma_start(out=out[:, :], in_=acc[:, :])
    nc.compile()
    return nc


def _get_built(dh=DH, chunks=CHUNKS):
    key = (dh, tuple(chunks))
    if key not in _CACHE:
        _CACHE[key] = _build_nc(dh, chunks)
    return _CACHE[key]


def _pack_keys(logits):
    """f32 logits [.., C, D, H, W] -> int16 keys, class id in low 4 bits.

    key = round((v+21)*64)*16 + c: monotone in v (grid 2^-6), so per-voxel
    argmax_c v_c == argmax_c key_c up to quantization ties (resolved to the
    larger class id). v in (-11, 11) -> key in (10240, 32767]: int16-exact.
    """
    q = np.rint((logits + np.float32(21.0)) * np.float32(64.0)).astype(np.int16)
    q <<= 4
    q |= np.arange(C, dtype=np.int16).reshape(C, 1, 1, 1)
    return q


def _shard_inputs(logits, target):
    """FULL inputs -> list of 8 per-core in_maps (b-major, then depth chunk).

    Host prep: zero-pad depth, pack logits to int16 keys as [H,C,dh,W],
    cast target to f16 as [H,dh,W] (partition dim first, depth*width
    contiguous per class for 2KB+ DMA runs).
    """
    keys = _pack_keys(np.asarray(logits, np.float32))
    lp = np.zeros((B, C, D + 2, H, W), np.int16)
    lp[:, :, 1:-1] = keys
    tp = np.zeros((B, D + 2, H, PW0), np.float16)
    tp[:, 1:-1, :, 2:130] = np.asarray(target, np.int32)[:, 0].astype(np.float16)
    wm = _make_wmats()
    maps = []
    for b in range(B):
        for j in range(D // DSH):
            s = j * DSH
            maps.append({
                "keys": np.ascontiguousarray(
                    lp[b, :, s : s + DH].transpose(2, 0, 1, 3)),
                "target": np.ascontiguousarray(
                    tp[b, s : s + DH].transpose(1, 0, 2)),
                "wmats": wm,
            })
    return maps


def kernel(logits: np.ndarray, target: np.ndarray) -> np.ndarray:
    from concourse.bass_utils import run_bass_kernel_spmd

    nc = _get_built()
    maps = _shard_inputs(np.asarray(logits), np.asarray(target))
    res = run_bass_kernel_spmd(nc, maps, list(range(N_CORES))).results
    total = 0.0
    for r in res:
        total += np.asarray(r["out"], np.float64).sum()
    loss = total / (576.0 * B * D * H * W)
    return np.float32(loss)


# ---------------- numpy reference for one shard (testing only) ----------------

def shard_partial_np(lg, tgt):
    """lg [C,dh,H,W] float (already +halo, zero-padded), tgt [dh,H,W] int.
    Returns sum over interior slices of 576*(pb-tb)^2*w^2."""
    pred = np.argmax(lg, axis=0).astype(np.float32)
    t = tgt.astype(np.float32)

    def S(x):
        xp = np.pad(x, ((0, 0), (1, 1), (1, 1)))
        s = xp[:, :, :-2] + 2 * xp[:, :, 1:-1] + xp[:, :, 2:]
        s = s[:, :-2, :] + 2 * s[:, 1:-1, :] + s[:, 2:, :]
        return s[:-2] + 2 * s[1:-1] + s[2:]

    Av = 32 * pred[1:-1] - S(pred)
    Bv = 32 * t[1:-1] - S(t)
    w = np.where((tgt[1:-1] < 2) | (tgt[1:-1] == 4), 1.0, 3.0).astype(np.float32)
    e = (np.abs(Av) - np.abs(Bv)) * w
    return float(np.sum((e * e).astype(np.float64)))
